# revision 8
# baseline (speedup 1.0000x reference)
"""Causal self-attention (B=2, T=2048, D=2048, 16 heads) on 8 NeuronCores.

Tensor-parallel over heads: core c owns heads {2c, 2c+1}. Each core computes
its heads' Q/K/V projections, causal attention, and a partial output
projection (row-parallel); the host sums the 8 partials.

Layout strategy (all fp32, matmuls in fp32r at full PE rate):
  - Host passes x transposed per batch: xT [B, D, T].
  - QKV weights pre-transposed/sliced on host: wqkvT [D, 768]
    (cols = [q_h0, q_h1, k_h0, k_h1, v_both(256)]... actually q|k|v blocks).
  - Q^T, K^T computed as [head_dim, T] tiles directly (natural lhsT/rhs form
    for S^T = K Q^T); V computed as [T, head_dim].
  - Attention works on S^T [key, query] tiles: causal mask is added for
    diagonal blocks, exp runs on ACT with the 1/sqrt(d) scale folded in,
    PV accumulates out^T [head_dim, T] in PSUM. Softmax denominator comes
    from a DVE accumulation + an M=1 ones-matmul (partition reduction),
    broadcast back via a K=1 matmul.
  - Output projection consumes out^T tiles as lhsT against woT [256, D].
"""
import numpy as np
from contextlib import ExitStack

import concourse.bass as bass
import concourse.tile as tile
from concourse import bacc
from concourse import mybir
from concourse.bass_utils import run_bass_kernel_spmd

f32 = mybir.dt.float32
f32r = mybir.dt.float32r

B, T, D = 2, 2048, 2048
H, HD = 16, 128
N_CORES = 8
NH = H // N_CORES            # heads per core = 2
SCALE = float(HD) ** -0.5    # 1/sqrt(128)
NEG = -1.0e9

DT = D // 128                # 16 D-tiles (contraction)
CH = 256                     # token chunk for QKV projection
NCH = T // CH                # 8 chunks per batch
TT = T // 128                # 16 token tiles per batch
QB = 512                     # query block for attention / feature block
NQB = T // QB                # 4


def _body(ctx, tc, xT, wqkvT, woT, mask, y):
    nc = tc.nc

    singles = ctx.enter_context(tc.tile_pool(name="singles", bufs=1))
    wqkv_sb = singles.tile([128, DT, 3 * NH * HD], f32r)
    nc.sync.dma_start(out=wqkv_sb, in_=wqkvT.rearrange("(n p) f -> p n f", p=128))
    wo_sb = singles.tile([128, NH, D], f32r)
    nc.sync.dma_start(out=wo_sb, in_=woT.rearrange("(n p) e -> p n e", p=128))
    mask_sb = singles.tile([128, QB // 128, QB], f32)
    nc.sync.dma_start(out=mask_sb, in_=mask.rearrange("(n p) q -> p n q", p=128))
    # Memset doesn't support f32r; stage in f32 and convert via DVE copy.
    ones_col_f = singles.tile([128, 1], f32)
    nc.vector.memset(ones_col_f, 1.0)
    ones_col = singles.tile([128, 1], f32r)
    nc.vector.tensor_copy(ones_col, ones_col_f)
    ones_row_f = singles.tile([1, 128], f32)
    nc.vector.memset(ones_row_f, 1.0)
    ones_row = singles.tile([1, 128], f32r)
    nc.vector.tensor_copy(ones_row, ones_row_f)

    perbatch = ctx.enter_context(tc.tile_pool(name="perbatch", bufs=1))
    xpool = ctx.enter_context(tc.tile_pool(name="xpool", bufs=2))
    attsb = ctx.enter_context(tc.tile_pool(name="attsb", bufs=3))
    smallsb = ctx.enter_context(tc.tile_pool(name="smallsb", bufs=2))
    ysb_pool = ctx.enter_context(tc.tile_pool(name="ysb", bufs=3))
    # Single PSUM pool, 4 tags x 2 bufs x 1 bank = exactly 8 banks.
    ps = ctx.enter_context(tc.tile_pool(name="ps", bufs=2, space="PSUM"))

    for b in range(B):
        qt_sb = perbatch.tile([128, NH, T], f32r, tag="qt")    # Q^T per head
        kt_sb = perbatch.tile([128, NH, T], f32r, tag="kt")    # K^T per head
        v_sb = perbatch.tile([128, NH, TT, HD], f32r, tag="v")  # V per head
        outT_sb = perbatch.tile([128, NH, T], f32r, tag="outT")

        # ---- Phase 1: QKV projections ----
        # One accumulation group at a time per PSUM bank: iterate the six
        # outputs (q_h0, q_h1, k_h0, k_h1, v_tt0, v_tt1) as sequential groups
        # sharing the "A" tag (2 bufs), each accumulating over all 16 D-tiles.
        xT_b = xT[b].rearrange("(n p) t -> p n t", p=128)
        groups = [("q", 0), ("q", 1), ("k", 0), ("k", 1)] + \
                 [("v", tt) for tt in range(CH // 128)]
        for ci in range(NCH):
            xch = xpool.tile([128, DT, CH], f32r, tag="xch")
            nc.sync.dma_start(out=xch, in_=xT_b[:, :, ci * CH:(ci + 1) * CH])
            for kind, idx in groups:
                acc = ps.tile([128, CH], f32, tag="A")
                for di in range(DT):
                    if kind == "q":
                        lhsT = wqkv_sb[:, di, idx * HD:(idx + 1) * HD]
                        rhs = xch[:, di, :]
                    elif kind == "k":
                        lhsT = wqkv_sb[:, di, (NH + idx) * HD:(NH + idx + 1) * HD]
                        rhs = xch[:, di, :]
                    else:
                        lhsT = xch[:, di, idx * 128:(idx + 1) * 128]
                        rhs = wqkv_sb[:, di, 2 * NH * HD:3 * NH * HD]
                    nc.tensor.matmul(acc, lhsT=lhsT,
                                     rhs=rhs,
                                     start=(di == 0), stop=(di == DT - 1))
                if kind == "q":
                    nc.vector.tensor_copy(
                        qt_sb[:, idx, ci * CH:(ci + 1) * CH], acc)
                elif kind == "k":
                    nc.scalar.copy(kt_sb[:, idx, ci * CH:(ci + 1) * CH], acc)
                else:
                    for h in range(NH):
                        nc.vector.tensor_copy(
                            v_sb[:, h, ci * (CH // 128) + idx, :],
                            acc[:, h * HD:(h + 1) * HD])

        # ---- Phase 2: causal attention per head ----
        for h in range(NH):
            for qb in range(NQB):
                nk = (qb + 1) * QB // 128
                o_ps = ps.tile([128, QB], f32, tag="A")
                den = attsb.tile([128, QB], f32r, tag="den")
                q_slice = qt_sb[:, h, qb * QB:(qb + 1) * QB]
                for kt in range(nk):
                    s_ps = ps.tile([128, QB], f32, tag="B")
                    nc.tensor.matmul(
                        s_ps,
                        lhsT=kt_sb[:, h, kt * 128:(kt + 1) * 128],
                        rhs=q_slice, start=True, stop=True)
                    k_rel = kt * 128 - qb * QB
                    if k_rel >= 0:
                        nc.vector.tensor_add(s_ps, s_ps, mask_sb[:, k_rel // 128, :])
                    pt = attsb.tile([128, QB], f32r, tag="pt")
                    nc.scalar.activation(pt, s_ps,
                                         mybir.ActivationFunctionType.Exp,
                                         scale=SCALE)
                    nc.tensor.matmul(
                        o_ps, lhsT=v_sb[:, h, kt, :],
                        rhs=pt,
                        start=(kt == 0), stop=(kt == nk - 1))
                    if kt == 0:
                        nc.vector.tensor_copy(den, pt)
                    else:
                        nc.vector.tensor_add(den, den, pt)
                den_ps = ps.tile([1, QB], f32, tag="C")
                nc.tensor.matmul(den_ps, lhsT=ones_col,
                                 rhs=den, start=True, stop=True)
                recip = smallsb.tile([1, QB], f32r, tag="rcp")
                nc.vector.reciprocal(recip, den_ps)
                rb_ps = ps.tile([128, QB], f32, tag="C")
                nc.tensor.matmul(rb_ps, lhsT=ones_row,
                                 rhs=recip, start=True, stop=True)
                osl = outT_sb[:, h, qb * QB:(qb + 1) * QB]
                nc.scalar.copy(osl, o_ps)
                nc.vector.tensor_mul(osl, osl, rb_ps)

        # ---- Phase 3: output projection (partial sums over local heads) ----
        for ti in range(TT):
            for eb in range(D // QB):
                y_ps = ps.tile([128, QB], f32, tag="B")
                for h in range(NH):
                    nc.tensor.matmul(
                        y_ps,
                        lhsT=outT_sb[:, h, ti * 128:(ti + 1) * 128],
                        rhs=wo_sb[:, h, eb * QB:(eb + 1) * QB],
                        start=(h == 0), stop=(h == NH - 1))
                y_tile = ysb_pool.tile([128, QB], f32, tag="yt")
                if (ti + eb) % 2 == 0:
                    nc.vector.tensor_copy(y_tile, y_ps)
                else:
                    nc.scalar.copy(y_tile, y_ps)
                nc.sync.dma_start(
                    out=y[b * T + ti * 128:b * T + (ti + 1) * 128,
                          eb * QB:(eb + 1) * QB],
                    in_=y_tile)


_NC_CACHE = None


def build_bass(do_compile=True):
    global _NC_CACHE
    if _NC_CACHE is not None:
        return _NC_CACHE
    nc = bacc.Bacc()
    xT = nc.declare_dram_parameter("xT", [B, D, T], f32r, isOutput=False)
    wqkvT = nc.declare_dram_parameter("wqkvT", [D, 3 * NH * HD], f32r, isOutput=False)
    woT = nc.declare_dram_parameter("woT", [NH * HD, D], f32r, isOutput=False)
    mask = nc.declare_dram_parameter("mask", [QB, QB], f32, isOutput=False)
    y = nc.declare_dram_parameter("y", [B * T, D], f32, isOutput=True)
    with tile.TileContext(nc) as tc:
        with ExitStack() as ctx:
            with nc.allow_low_precision(
                    reason="fp32r tiles feed full-rate PE matmuls; storage is "
                           "still 32-bit"):
                _body(ctx, tc, xT, wqkvT, woT, mask, y[:, :])
    if do_compile:
        nc.compile()
    _NC_CACHE = nc
    return nc


def shard_inputs(x, W_qkv, W_out):
    x = np.asarray(x, dtype=np.float32)
    W_qkv = np.asarray(W_qkv, dtype=np.float32)
    W_out = np.asarray(W_out, dtype=np.float32)

    xT = np.ascontiguousarray(x.transpose(0, 2, 1))          # [B, D, T]
    i = np.arange(QB)
    mask = np.where(i[:, None] <= i[None, :], 0.0, NEG).astype(np.float32)

    in_maps = []
    for c in range(N_CORES):
        r0 = c * NH * HD
        r1 = r0 + NH * HD
        wq = W_qkv[r0:r1].T                                   # [D, 256]
        wk = W_qkv[D + r0:D + r1].T
        wv = W_qkv[2 * D + r0:2 * D + r1].T
        wqkvT = np.ascontiguousarray(np.concatenate([wq, wk, wv], axis=1))
        woT = np.ascontiguousarray(W_out[:, r0:r1].T)         # [256, D]
        in_maps.append({"xT": xT, "wqkvT": wqkvT, "woT": woT, "mask": mask})
    return in_maps


def run(x, W_qkv, W_out, trace=False):
    nc = build_bass()
    in_maps = shard_inputs(x, W_qkv, W_out)
    res = run_bass_kernel_spmd(nc, in_maps, list(range(N_CORES)), trace=trace)
    parts = np.stack([r["y"] for r in res.results])           # [8, B*T, D]
    y = parts.sum(axis=0, dtype=np.float64).astype(np.float32)
    return y.reshape(B, T, D), res


def kernel(x, W_qkv, W_out):
    y, _ = run(x, W_qkv, W_out, trace=False)
    return y


# revision 22
# speedup vs baseline: 1.1875x; 1.1875x over previous
"""Causal self-attention (B=2, T=2048, D=2048, 16 heads) on 8 NeuronCores.

Tensor-parallel over heads: core c owns heads {2c, 2c+1}. Each core computes
its heads' Q/K/V projections, causal attention, and a partial output
projection (row-parallel); the host sums the 8 partials.

Layout strategy (fp32 storage, matmuls in fp32r at full PE rate):
  - Host passes x transposed per batch: xT [B, D, T].
  - Q^T, K^T computed as [head_dim, T] tiles (natural lhsT/rhs form for
    S^T = K Q^T); V computed as [T, head_dim].
  - Attention works on S^T [key, query] tiles, two k-tiles at a time:
    a [128,128] triangle mask is added on diagonal tiles, exp runs on ACT
    with the 1/sqrt(d) scale folded in (skipping fully-masked columns,
    which GpSimd memsets to zero), PV accumulates out^T [head_dim, T] in
    PSUM. The softmax denominator accumulates on GpSimd (otherwise idle);
    an M=1 ones-matmul reduces it across partitions and a K=1 ones-matmul
    broadcasts the reciprocal back.
  - Output projection is interleaved per query-block to spread its DMA.
"""
import numpy as np
from contextlib import ExitStack

import concourse.bass as bass
import concourse.tile as tile
from concourse import bacc
from concourse import mybir
from concourse.bass_utils import run_bass_kernel_spmd

f32 = mybir.dt.float32
f32r = mybir.dt.float32r

B, T, D = 2, 2048, 2048
H, HD = 16, 128
N_CORES = 8
NH = H // N_CORES            # heads per core = 2
SCALE = float(HD) ** -0.5    # 1/sqrt(128)
NEG = -1.0e9

DT = D // 128                # 16 D-tiles (contraction)
CH = 256                     # token chunk for QKV projection
NCH = T // CH                # 8 chunks per batch
TT = T // 128                # 16 token tiles per batch
QB = 512                     # query block for attention / feature block
NQB = T // QB                # 4


def _body(ctx, tc, xT, wqkvT, woT, mask, y):
    nc = tc.nc

    singles = ctx.enter_context(tc.tile_pool(name="singles", bufs=1))
    wqkv_sb = singles.tile([128, DT, 3 * NH * HD], f32r)
    wqkvT_r = wqkvT.rearrange("(n p) f -> p n f", p=128)
    # wo/mask are loaded later (first needed at attention/out-proj time).
    wo_sb = singles.tile([128, NH, D], f32r)
    mask_sb = singles.tile([128, 128], f32)
    # Memset doesn't support f32r; stage in f32 and convert via DVE copy.
    ones_col_f = singles.tile([128, 1], f32)
    nc.vector.memset(ones_col_f, 1.0)
    ones_col = singles.tile([128, 1], f32r)
    nc.vector.tensor_copy(ones_col, ones_col_f)
    ones_row_f = singles.tile([1, 128], f32)
    nc.vector.memset(ones_row_f, 1.0)
    ones_row = singles.tile([1, 128], f32r)
    nc.vector.tensor_copy(ones_row, ones_row_f)
    zeros_f = singles.tile([128, 128], f32)
    nc.vector.memset(zeros_f, 0.0)
    zeros_sb = singles.tile([128, 128], f32r)
    nc.vector.tensor_copy(zeros_sb, zeros_f)

    perbatch = ctx.enter_context(tc.tile_pool(name="perbatch", bufs=1))
    xpool = ctx.enter_context(tc.tile_pool(name="xpool", bufs=2))
    attsb = ctx.enter_context(tc.tile_pool(name="attsb", bufs=3))
    densb = ctx.enter_context(tc.tile_pool(name="densb", bufs=2))
    smallsb = ctx.enter_context(tc.tile_pool(name="smallsb", bufs=2))
    ysb_pool = ctx.enter_context(tc.tile_pool(name="ysb", bufs=3))
    # PSUM budget: A{qkv acc, y} 1 bank x2 + B{s2 pairs} 2 banks x2 +
    # O{attention out} 1 bank x1 + C{den, rb} 1 bank x1 = 8 banks.
    ps = ctx.enter_context(tc.tile_pool(name="ps", bufs=2, space="PSUM"))

    for b in range(B):
        qt_sb = perbatch.tile([128, NH, T], f32r, tag="qt")    # Q^T per head
        kt_sb = perbatch.tile([128, NH, T], f32r, tag="kt")    # K^T per head
        v_sb = perbatch.tile([128, NH, TT, HD], f32r, tag="v")  # V per head
        outT_sb = perbatch.tile([128, NH, T], f32r, tag="outT")

        # QKV projection for one token chunk: six sequential accumulation
        # groups (q_h0, q_h1, k_h0, k_h1, v_tt0, v_tt1) over all 16 D-tiles.
        xT_b = xT[b].rearrange("(n p) t -> p n t", p=128)
        groups = [("q", 0), ("q", 1), ("k", 0), ("k", 1)] + \
                 [("v", tt) for tt in range(CH // 128)]

        def qkv_chunk(ci):
            xch = xpool.tile([128, DT, CH], f32r, tag="xch")
            src = xT_b[:, :, ci * CH:(ci + 1) * CH]
            if b == 0 and ci == 0:
                # Critical first loads: interleave the x chunk and the qkv
                # weights in the order the first accumulation group consumes
                # them, so the first matmul starts after ~1.5 MB of DMA.
                nc.sync.dma_start(out=xch[:, 0:4, :], in_=src[:, 0:4, :])
                nc.sync.dma_start(out=wqkv_sb[:, :, 0:128],
                                  in_=wqkvT_r[:, :, 0:128])
                nc.sync.dma_start(out=xch[:, 4:, :], in_=src[:, 4:, :])
                for f0, f1 in ((128, 256), (256, 384), (384, 512), (512, 768)):
                    nc.sync.dma_start(out=wqkv_sb[:, :, f0:f1],
                                      in_=wqkvT_r[:, :, f0:f1])
            else:
                nc.sync.dma_start(out=xch, in_=src)
            for kind, idx in groups:
                acc = ps.tile([128, CH], f32, tag="A")
                for di in range(DT):
                    if kind == "q":
                        lhsT = wqkv_sb[:, di, idx * HD:(idx + 1) * HD]
                        rhs = xch[:, di, :]
                    elif kind == "k":
                        lhsT = wqkv_sb[:, di, (NH + idx) * HD:(NH + idx + 1) * HD]
                        rhs = xch[:, di, :]
                    else:
                        lhsT = xch[:, di, idx * 128:(idx + 1) * 128]
                        rhs = wqkv_sb[:, di, 2 * NH * HD:3 * NH * HD]
                    nc.tensor.matmul(acc, lhsT=lhsT, rhs=rhs,
                                     start=(di == 0), stop=(di == DT - 1))
                cols = slice(ci * CH, (ci + 1) * CH)
                if kind == "q":
                    nc.vector.tensor_copy(qt_sb[:, idx, cols], acc)
                elif kind == "k":
                    nc.scalar.copy(kt_sb[:, idx, cols], acc)
                else:
                    nc.vector.tensor_copy(
                        v_sb[:, :, ci * (CH // 128) + idx, :],
                        acc.rearrange("p (h d) -> p h d", h=NH))

        def out_proj(qb):
            for ti in range(qb * (QB // 128), (qb + 1) * (QB // 128)):
                for eb in range(D // QB):
                    y_ps = ps.tile([128, QB], f32, tag="A")
                    for h in range(NH):
                        nc.tensor.matmul(
                            y_ps,
                            lhsT=outT_sb[:, h, ti * 128:(ti + 1) * 128],
                            rhs=wo_sb[:, h, eb * QB:(eb + 1) * QB],
                            start=(h == 0), stop=(h == NH - 1))
                    y_tile = ysb_pool.tile([128, QB], f32, tag="yt")
                    if (ti + eb) % 2 == 0:
                        nc.vector.tensor_copy(y_tile, y_ps)
                    else:
                        nc.scalar.copy(y_tile, y_ps)
                    nc.sync.dma_start(
                        out=y[b * T + ti * 128:b * T + (ti + 1) * 128,
                              eb * QB:(eb + 1) * QB],
                        in_=y_tile)

        def attention(qb):
            for h in range(NH):
                nk = (qb + 1) * QB // 128
                o_ps = ps.tile([128, QB], f32, tag="O", bufs=1)
                # Double accumulator: one wide DVE op per k-tile pair; the
                # two halves are folded by the PE ones-matmul reduction.
                den2 = densb.tile([128, 2, QB], f32r, tag="den")
                q_slice = qt_sb[:, h, qb * QB:(qb + 1) * QB]
                for p in range(nk // 2):
                    s2 = ps.tile([128, 2, QB], f32, tag="B")
                    pt2 = attsb.tile([128, 2, QB], f32r, tag="pt")
                    for j in range(2):
                        kt = 2 * p + j
                        nc.tensor.matmul(
                            s2[:, j, :],
                            lhsT=kt_sb[:, h, kt * 128:(kt + 1) * 128],
                            rhs=q_slice, start=True, stop=True)
                    k_rel0 = (2 * p) * 128 - qb * QB
                    diag = k_rel0 >= 0
                    if diag:
                        # Diagonal pair: triangle mask, then per-subtile exp
                        # restricted to the valid column range. Columns below
                        # the diagonal are never read downstream (PV and den
                        # are restricted the same way), so no memset needed.
                        for j in range(2):
                            kr = k_rel0 + j * 128
                            nc.vector.tensor_add(
                                s2[:, j, kr:kr + 128], s2[:, j, kr:kr + 128],
                                mask_sb)
                            nc.scalar.activation(
                                pt2[:, j, kr:], s2[:, j, kr:],
                                mybir.ActivationFunctionType.Exp, scale=SCALE)
                    else:
                        nc.scalar.activation(
                            pt2, s2, mybir.ActivationFunctionType.Exp,
                            scale=SCALE)
                    for j in range(2):
                        kt = 2 * p + j
                        kr = max(k_rel0 + j * 128, 0) if diag else 0
                        nc.tensor.matmul(
                            o_ps[:, kr:], lhsT=v_sb[:, h, kt, :],
                            rhs=pt2[:, j, kr:],
                            start=(kt == 0), stop=(kt == nk - 1))
                    if p == 0:
                        if diag:
                            # qb == 0: j=0 is full width (kr=0); j=1 starts
                            # at column 128 — zero-fill the gap so the PE
                            # fold below reads initialized data.
                            nc.gpsimd.tensor_copy(den2[:, 0, :], pt2[:, 0, :])
                            nc.gpsimd.tensor_copy(den2[:, 1, 128:],
                                                  pt2[:, 1, 128:])
                            nc.vector.tensor_copy(den2[:, 1, 0:128], zeros_sb)
                        else:
                            # 1-input copy runs near line-rate on GpSimd
                            # (P12), keeping the chain head off the busy DVE.
                            nc.gpsimd.tensor_copy(den2, pt2)
                    elif diag:
                        for j in range(2):
                            kr = k_rel0 + j * 128
                            nc.vector.tensor_add(den2[:, j, kr:],
                                                 den2[:, j, kr:],
                                                 pt2[:, j, kr:])
                    else:
                        nc.vector.tensor_add(den2, den2, pt2)
                den_ps = ps.tile([1, QB], f32, tag="C", bufs=1)
                for j in range(2):
                    nc.tensor.matmul(den_ps, lhsT=ones_col, rhs=den2[:, j, :],
                                     start=(j == 0), stop=(j == 1))
                recip = smallsb.tile([1, QB], f32r, tag="rcp")
                nc.vector.reciprocal(recip, den_ps)
                rb_ps = ps.tile([128, QB], f32, tag="C", bufs=1)
                nc.tensor.matmul(rb_ps, lhsT=ones_row, rhs=recip,
                                 start=True, stop=True)
                osl = outT_sb[:, h, qb * QB:(qb + 1) * QB]
                nc.scalar.copy(osl, o_ps)
                nc.vector.tensor_mul(osl, osl, rb_ps)

        # Interleave: attention for query block qb only needs the first
        # 2*qb+2 QKV chunks, so QKV (pure PE) overlaps attention's DVE/ACT
        # load; the output projection lags one block so the denominator
        # chain of block qb overlaps block qb+1's k-loop.
        for c in range(NQB):
            qkv_chunk(2 * c)
            qkv_chunk(2 * c + 1)
            if b == 0 and c == 0:
                nc.sync.dma_start(out=mask_sb, in_=mask[:, :])
            if b == 0 and c == 1:
                nc.sync.dma_start(
                    out=wo_sb, in_=woT.rearrange("(n p) e -> p n e", p=128))
            attention(c)
            if c > 0:
                out_proj(c - 1)
        out_proj(NQB - 1)


_NC_CACHE = {}


def build_bass(do_compile=True):
    if do_compile in _NC_CACHE:
        return _NC_CACHE[do_compile]
    nc = bacc.Bacc()
    xT = nc.declare_dram_parameter("xT", [B, D, T], f32r, isOutput=False)
    wqkvT = nc.declare_dram_parameter("wqkvT", [D, 3 * NH * HD], f32r, isOutput=False)
    woT = nc.declare_dram_parameter("woT", [NH * HD, D], f32r, isOutput=False)
    mask = nc.declare_dram_parameter("mask", [128, 128], f32, isOutput=False)
    y = nc.declare_dram_parameter("y", [B * T, D], f32, isOutput=True)
    with tile.TileContext(nc) as tc:
        with ExitStack() as ctx:
            with nc.allow_low_precision(
                    reason="fp32r tiles feed full-rate PE matmuls; storage is "
                           "still 32-bit"):
                _body(ctx, tc, xT, wqkvT, woT, mask, y[:, :])
    if do_compile:
        nc.compile()
    _NC_CACHE[do_compile] = nc
    return nc


def shard_inputs(x, W_qkv, W_out):
    x = np.asarray(x, dtype=np.float32)
    W_qkv = np.asarray(W_qkv, dtype=np.float32)
    W_out = np.asarray(W_out, dtype=np.float32)

    xT = np.ascontiguousarray(x.transpose(0, 2, 1))          # [B, D, T]
    i = np.arange(128)
    mask = np.where(i[:, None] <= i[None, :], 0.0, NEG).astype(np.float32)

    in_maps = []
    for c in range(N_CORES):
        r0 = c * NH * HD
        r1 = r0 + NH * HD
        wq = W_qkv[r0:r1].T                                   # [D, 256]
        wk = W_qkv[D + r0:D + r1].T
        wv = W_qkv[2 * D + r0:2 * D + r1].T
        wqkvT = np.ascontiguousarray(np.concatenate([wq, wk, wv], axis=1))
        woT = np.ascontiguousarray(W_out[:, r0:r1].T)         # [256, D]
        in_maps.append({"xT": xT, "wqkvT": wqkvT, "woT": woT, "mask": mask})
    return in_maps


def run(x, W_qkv, W_out, trace=False):
    nc = build_bass()
    in_maps = shard_inputs(x, W_qkv, W_out)
    res = run_bass_kernel_spmd(nc, in_maps, list(range(N_CORES)), trace=trace)
    parts = np.stack([r["y"] for r in res.results])           # [8, B*T, D]
    y = parts.sum(axis=0, dtype=np.float64).astype(np.float32)
    return y.reshape(B, T, D), res


def kernel(x, W_qkv, W_out):
    y, _ = run(x, W_qkv, W_out, trace=False)
    return y


# revision 28
# speedup vs baseline: 1.3068x; 1.1004x over previous
"""Causal self-attention (B=2, T=2048, D=2048, 16 heads) on 8 NeuronCores.

Tensor-parallel over heads: core c owns heads {2c, 2c+1}. Each core computes
its heads' Q/K/V projections, causal attention, and a partial output
projection (row-parallel); the host sums the 8 partials.

Layout strategy (fp32 storage, matmuls in fp32r at full PE rate):
  - Host passes x transposed per batch: xT [B, D, T].
  - Q^T, K^T computed as [head_dim, T] tiles (natural lhsT/rhs form for
    S^T = K Q^T); V computed as [T, head_dim].
  - Attention works on S^T [key, query] tiles, two k-tiles at a time:
    a [128,128] triangle mask is added on diagonal tiles, exp runs on ACT
    with the 1/sqrt(d) scale folded in, PV accumulates out^T [head_dim, T]
    in PSUM. On diagonal sub-tiles exp/PV/den are restricted to the valid
    column range so fully-masked columns are never computed or read.
    The softmax denominator accumulates as one wide DVE op per pair into a
    double accumulator (chain head on GpSimd); two accumulating M=1
    ones-matmuls fold + reduce it across partitions and a K=1 ones-matmul
    broadcasts the reciprocal back.
  - QKV runs interleaved with attention (block qb needs only the first
    2qb+2 token chunks) and the output projection lags one query block,
    spreading its DMA and overlapping the denominator tail.
"""
import numpy as np
from contextlib import ExitStack

import concourse.bass as bass
import concourse.tile as tile
from concourse import bacc
from concourse import mybir
from concourse.bass_utils import run_bass_kernel_spmd

f32 = mybir.dt.float32
f32r = mybir.dt.float32r

B, T, D = 2, 2048, 2048
H, HD = 16, 128
N_CORES = 8
NH = H // N_CORES            # heads per core = 2
SCALE = float(HD) ** -0.5    # 1/sqrt(128)
NEG = -1.0e9

DT = D // 128                # 16 D-tiles (contraction)
CH = 256                     # token chunk for QKV projection
NCH = T // CH                # 8 chunks per batch
TT = T // 128                # 16 token tiles per batch
QB = 512                     # query block for attention / feature block
NQB = T // QB                # 4


def _body(ctx, tc, xT, wqkvT, woT, mask, y):
    nc = tc.nc

    singles = ctx.enter_context(tc.tile_pool(name="singles", bufs=1))
    wqkv_sb = singles.tile([128, DT, 3 * NH * HD], f32r)
    wqkvT_r = wqkvT.rearrange("(n p) f -> p n f", p=128)
    # wo/mask are loaded later (first needed at attention/out-proj time).
    wo_sb = singles.tile([128, NH, D], f32r)
    mask_sb = singles.tile([128, 128], f32)
    # Memset doesn't support f32r; stage in f32 and convert via DVE copy.
    ones_col_f = singles.tile([128, 1], f32)
    nc.vector.memset(ones_col_f, 1.0)
    ones_col = singles.tile([128, 1], f32r)
    nc.vector.tensor_copy(ones_col, ones_col_f)
    ones_row_f = singles.tile([1, 128], f32)
    nc.vector.memset(ones_row_f, 1.0)
    ones_row = singles.tile([1, 128], f32r)
    nc.vector.tensor_copy(ones_row, ones_row_f)
    zeros_f = singles.tile([128, 128], f32)
    nc.vector.memset(zeros_f, 0.0)
    zeros_sb = singles.tile([128, 128], f32r)
    nc.vector.tensor_copy(zeros_sb, zeros_f)

    perbatch = ctx.enter_context(tc.tile_pool(name="perbatch", bufs=1))
    xpool = ctx.enter_context(tc.tile_pool(name="xpool", bufs=2))
    attsb = ctx.enter_context(tc.tile_pool(name="attsb", bufs=4))
    densb = ctx.enter_context(tc.tile_pool(name="densb", bufs=2))
    smallsb = ctx.enter_context(tc.tile_pool(name="smallsb", bufs=2))
    ysb_pool = ctx.enter_context(tc.tile_pool(name="ysb", bufs=7))
    # PSUM budget: A{qkv acc, y} 1 bank x2 + B{s2 pairs} 2 banks x2 +
    # O{attention out} 1 bank x1 + C{den, rb} 1 bank x1 = 8 banks.
    ps = ctx.enter_context(tc.tile_pool(name="ps", bufs=2, space="PSUM"))

    for b in range(B):
        qt_sb = perbatch.tile([128, NH, T], f32r, tag="qt")    # Q^T per head
        kt_sb = perbatch.tile([128, NH, T], f32r, tag="kt")    # K^T per head
        v_sb = perbatch.tile([128, NH, TT, HD], f32r, tag="v")  # V per head
        outT_sb = perbatch.tile([128, NH, T], f32r, tag="outT")

        # QKV projection for one token chunk: six sequential accumulation
        # groups (q_h0, q_h1, k_h0, k_h1, v_tt0, v_tt1) over all 16 D-tiles.
        xT_b = xT[b].rearrange("(n p) t -> p n t", p=128)
        groups = [("q", 0), ("q", 1), ("k", 0), ("k", 1)] + \
                 [("v", tt) for tt in range(CH // 128)]

        def qkv_chunk(ci):
            xch = xpool.tile([128, DT, CH], f32r, tag="xch")
            src = xT_b[:, :, ci * CH:(ci + 1) * CH]
            if b == 0 and ci == 0:
                # Critical first loads: interleave the x chunk and the qkv
                # weights in the order the first accumulation group consumes
                # them, so the first matmul starts after ~1.5 MB of DMA.
                nc.sync.dma_start(out=xch[:, 0:4, :], in_=src[:, 0:4, :])
                nc.sync.dma_start(out=wqkv_sb[:, :, 0:128],
                                  in_=wqkvT_r[:, :, 0:128])
                nc.sync.dma_start(out=xch[:, 4:, :], in_=src[:, 4:, :])
                for f0, f1 in ((128, 256), (256, 384), (384, 512), (512, 768)):
                    nc.sync.dma_start(out=wqkv_sb[:, :, f0:f1],
                                      in_=wqkvT_r[:, :, f0:f1])
            else:
                nc.sync.dma_start(out=xch, in_=src)
            for kind, idx in groups:
                acc = ps.tile([128, CH], f32, tag="A")
                for di in range(DT):
                    if kind == "q":
                        lhsT = wqkv_sb[:, di, idx * HD:(idx + 1) * HD]
                        rhs = xch[:, di, :]
                    elif kind == "k":
                        lhsT = wqkv_sb[:, di, (NH + idx) * HD:(NH + idx + 1) * HD]
                        rhs = xch[:, di, :]
                    else:
                        lhsT = xch[:, di, idx * 128:(idx + 1) * 128]
                        rhs = wqkv_sb[:, di, 2 * NH * HD:3 * NH * HD]
                    nc.tensor.matmul(acc, lhsT=lhsT, rhs=rhs,
                                     start=(di == 0), stop=(di == DT - 1))
                cols = slice(ci * CH, (ci + 1) * CH)
                if kind == "q":
                    nc.vector.tensor_copy(qt_sb[:, idx, cols], acc)
                elif kind == "k":
                    nc.scalar.copy(kt_sb[:, idx, cols], acc)
                else:
                    nc.vector.tensor_copy(
                        v_sb[:, :, ci * (CH // 128) + idx, :],
                        acc.rearrange("p (h d) -> p h d", h=NH))

        def out_proj(qb):
            for ti in range(qb * (QB // 128), (qb + 1) * (QB // 128)):
                for eb in range(D // QB):
                    y_ps = ps.tile([128, QB], f32, tag="A")
                    for h in range(NH):
                        nc.tensor.matmul(
                            y_ps,
                            lhsT=outT_sb[:, h, ti * 128:(ti + 1) * 128],
                            rhs=wo_sb[:, h, eb * QB:(eb + 1) * QB],
                            start=(h == 0), stop=(h == NH - 1))
                    y_tile = ysb_pool.tile([128, QB], f32, tag="yt")
                    if (ti + eb) % 2 == 0:
                        nc.vector.tensor_copy(y_tile, y_ps)
                    else:
                        nc.scalar.copy(y_tile, y_ps)
                    nc.sync.dma_start(
                        out=y[b * T + ti * 128:b * T + (ti + 1) * 128,
                              eb * QB:(eb + 1) * QB],
                        in_=y_tile)

        def attention(qb):
            for h in range(NH):
                nk = (qb + 1) * QB // 128
                o_ps = ps.tile([128, QB], f32, tag="O", bufs=1)
                # Double accumulator: one wide DVE op per k-tile pair; the
                # two halves are folded by the PE ones-matmul reduction.
                den2 = densb.tile([128, 2, QB], f32r, tag="den")
                q_slice = qt_sb[:, h, qb * QB:(qb + 1) * QB]
                for p in range(nk // 2):
                    s2 = ps.tile([128, 2, QB], f32, tag="B")
                    pt2 = attsb.tile([128, 2, QB], f32r, tag="pt")
                    for j in range(2):
                        kt = 2 * p + j
                        nc.tensor.matmul(
                            s2[:, j, :],
                            lhsT=kt_sb[:, h, kt * 128:(kt + 1) * 128],
                            rhs=q_slice, start=True, stop=True)
                    k_rel0 = (2 * p) * 128 - qb * QB
                    diag = k_rel0 >= 0
                    if diag:
                        # Diagonal pair: triangle mask, then per-subtile exp
                        # restricted to the valid column range. Columns below
                        # the diagonal are never read downstream (PV and den
                        # are restricted the same way), so no memset needed.
                        for j in range(2):
                            kr = k_rel0 + j * 128
                            nc.vector.tensor_add(
                                s2[:, j, kr:kr + 128], s2[:, j, kr:kr + 128],
                                mask_sb)
                            nc.scalar.activation(
                                pt2[:, j, kr:], s2[:, j, kr:],
                                mybir.ActivationFunctionType.Exp, scale=SCALE)
                    else:
                        nc.scalar.activation(
                            pt2, s2, mybir.ActivationFunctionType.Exp,
                            scale=SCALE)
                    for j in range(2):
                        kt = 2 * p + j
                        kr = max(k_rel0 + j * 128, 0) if diag else 0
                        nc.tensor.matmul(
                            o_ps[:, kr:], lhsT=v_sb[:, h, kt, :],
                            rhs=pt2[:, j, kr:],
                            start=(kt == 0), stop=(kt == nk - 1))
                    if p == 0:
                        if diag:
                            # qb == 0: j=0 is full width (kr=0); j=1 starts
                            # at column 128 — zero-fill the gap so the PE
                            # fold below reads initialized data.
                            nc.gpsimd.tensor_copy(den2[:, 0, :], pt2[:, 0, :])
                            nc.gpsimd.tensor_copy(den2[:, 1, 128:],
                                                  pt2[:, 1, 128:])
                            nc.vector.tensor_copy(den2[:, 1, 0:128], zeros_sb)
                        else:
                            # 1-input copy runs near line-rate on GpSimd
                            # (P12), keeping the chain head off the busy DVE.
                            nc.gpsimd.tensor_copy(den2, pt2)
                    elif diag:
                        for j in range(2):
                            kr = k_rel0 + j * 128
                            nc.vector.tensor_add(den2[:, j, kr:],
                                                 den2[:, j, kr:],
                                                 pt2[:, j, kr:])
                    else:
                        nc.vector.tensor_add(den2, den2, pt2)
                den_ps = ps.tile([1, QB], f32, tag="C", bufs=1)
                for j in range(2):
                    nc.tensor.matmul(den_ps, lhsT=ones_col, rhs=den2[:, j, :],
                                     start=(j == 0), stop=(j == 1))
                recip = smallsb.tile([1, QB], f32r, tag="rcp")
                nc.vector.reciprocal(recip, den_ps)
                rb_ps = ps.tile([128, QB], f32, tag="C", bufs=1)
                nc.tensor.matmul(rb_ps, lhsT=ones_row, rhs=recip,
                                 start=True, stop=True)
                osl = outT_sb[:, h, qb * QB:(qb + 1) * QB]
                nc.scalar.copy(osl, o_ps)
                nc.vector.tensor_mul(osl, osl, rb_ps)

        # Interleave: attention for query block qb only needs the first
        # 2*qb+2 QKV chunks, so QKV (pure PE) overlaps attention's DVE/ACT
        # load; the output projection lags one block so the denominator
        # chain of block qb overlaps block qb+1's k-loop.
        for c in range(NQB):
            qkv_chunk(2 * c)
            qkv_chunk(2 * c + 1)
            if b == 0 and c == 0:
                nc.sync.dma_start(out=mask_sb, in_=mask[:, :])
            if b == 0 and c == 1:
                nc.sync.dma_start(
                    out=wo_sb, in_=woT.rearrange("(n p) e -> p n e", p=128))
            attention(c)
            if c > 0:
                out_proj(c - 1)
        out_proj(NQB - 1)


_NC_CACHE = {}


def build_bass(do_compile=True):
    if do_compile in _NC_CACHE:
        return _NC_CACHE[do_compile]
    nc = bacc.Bacc()
    xT = nc.declare_dram_parameter("xT", [B, D, T], f32r, isOutput=False)
    wqkvT = nc.declare_dram_parameter("wqkvT", [D, 3 * NH * HD], f32r, isOutput=False)
    woT = nc.declare_dram_parameter("woT", [NH * HD, D], f32r, isOutput=False)
    mask = nc.declare_dram_parameter("mask", [128, 128], f32, isOutput=False)
    y = nc.declare_dram_parameter("y", [B * T, D], f32, isOutput=True)
    with tile.TileContext(nc) as tc:
        with ExitStack() as ctx:
            with nc.allow_low_precision(
                    reason="fp32r tiles feed full-rate PE matmuls; storage is "
                           "still 32-bit"):
                _body(ctx, tc, xT, wqkvT, woT, mask, y[:, :])
    if do_compile:
        nc.compile()
    _NC_CACHE[do_compile] = nc
    return nc


def shard_inputs(x, W_qkv, W_out):
    x = np.asarray(x, dtype=np.float32)
    W_qkv = np.asarray(W_qkv, dtype=np.float32)
    W_out = np.asarray(W_out, dtype=np.float32)

    xT = np.ascontiguousarray(x.transpose(0, 2, 1))          # [B, D, T]
    i = np.arange(128)
    mask = np.where(i[:, None] <= i[None, :], 0.0, NEG).astype(np.float32)

    in_maps = []
    for c in range(N_CORES):
        r0 = c * NH * HD
        r1 = r0 + NH * HD
        wq = W_qkv[r0:r1].T                                   # [D, 256]
        wk = W_qkv[D + r0:D + r1].T
        wv = W_qkv[2 * D + r0:2 * D + r1].T
        wqkvT = np.ascontiguousarray(np.concatenate([wq, wk, wv], axis=1))
        woT = np.ascontiguousarray(W_out[:, r0:r1].T)         # [256, D]
        in_maps.append({"xT": xT, "wqkvT": wqkvT, "woT": woT, "mask": mask})
    return in_maps


def run(x, W_qkv, W_out, trace=False):
    nc = build_bass()
    in_maps = shard_inputs(x, W_qkv, W_out)
    res = run_bass_kernel_spmd(nc, in_maps, list(range(N_CORES)), trace=trace)
    parts = np.stack([r["y"] for r in res.results])           # [8, B*T, D]
    y = parts.sum(axis=0, dtype=np.float64).astype(np.float32)
    return y.reshape(B, T, D), res


def kernel(x, W_qkv, W_out):
    y, _ = run(x, W_qkv, W_out, trace=False)
    return y
